# revision 9
# baseline (speedup 1.0000x reference)
"""Distributed CLIP-style batch contrastive loss on 8 Trainium2 NeuronCores.

Math (B=8192, D=256, temperature=0.07):
    tn = l2norm(text), gn = l2norm(graph)
    sim = tn @ gn.T / temp                                  [B, B]
    t2g = mean(LSE_row(sim)) - mean(diag(sim))
    g2t = mean(LSE_col(sim)) - mean(diag(sim))
    acc = mean(diag is row/col argmax)
    out = [t2g+g2t, t2g, g2t, t2g_acc, g2t_acc]

Since |sim| <= 1/temp = 14.29, exp(sim) never overflows f32, so LSE is
computed without max subtraction:  LSE = log(sum(exp(x/temp))).

Sharding: each core owns a 1024-row shard of both modalities.  It
normalizes its shards (bf16 output, f32 accumulation), transposes them to
d-major via the PE, and AllGathers both modalities.  Each core then
computes two [1024, 8192] similarity slabs with bf16 matmuls (f32 PSUM):
  slab1 = tn_k @ gn_all.T   -> row stats    (t2g direction)
  slab2 = gn_k @ tn_all.T   -> column stats (g2t direction)
Row-wise exp-sums ride the ScalarE activation accumulator; row maxes are
VectorE reductions straight out of PSUM.  The diagonal block values are
computed locally from the natural-layout shards (t_i . g_i), which keeps
the whole program independent of the core id (pure SPMD).  Per-core
partial sums are AllReduced and every core assembles the final [5] vector.
"""

import numpy as np

import concourse.bacc as bacc
import concourse.bass as bass
import concourse.mybir as mybir
import concourse.tile as tile
from concourse import masks
from concourse.bass_utils import run_bass_kernel_spmd

B = 8192
D = 256
NCORES = 8
BS = B // NCORES          # 1024 rows per core
MT = BS // 128            # 8 m-tiles per shard
NCHUNK = 512              # psum bank = 512 f32
NCH = B // NCHUNK         # 16 n-chunks per slab row-tile
TEMP = 0.07
INV_TEMP = 1.0 / TEMP

F32 = mybir.dt.float32
BF16 = mybir.dt.bfloat16
AX = mybir.AxisListType.X
OP = mybir.AluOpType
AF = mybir.ActivationFunctionType


def emit(ctx, tc, t_in, g_in, out5, mode="full"):
    nc = tc.nc
    rg = [list(range(NCORES))]

    consts = ctx.enter_context(tc.tile_pool(name="consts", bufs=1))
    ident_bf = consts.tile([128, 128], BF16, name="ident_bf")
    masks.make_identity(nc, ident_bf)
    ones_f32 = consts.tile([128, 1], F32, name="ones_f32")
    nc.vector.memset(ones_f32, 1.0)

    # persistent SBUF tensors
    pers = ctx.enter_context(tc.tile_pool(name="pers", bufs=1))
    tdm = [pers.tile([128, BS], BF16, name=f"tdm{h}") for h in range(2)]
    gdm = [pers.tile([128, BS], BF16, name=f"gdm{h}") for h in range(2)]
    tfull = [pers.tile([128, B], BF16, name=f"tfull{h}") for h in range(2)]
    gfull = [pers.tile([128, B], BF16, name=f"gfull{h}") for h in range(2)]
    dbuf = pers.tile([128, MT], F32, name="dbuf")        # raw diag dots
    partials = pers.tile([128, 8], F32, name="partials")  # lse1 lse2 d a1 a2

    # DRAM bounce buffers for collectives
    dram = ctx.enter_context(tc.tile_pool(name="dram", bufs=1, space="DRAM"))
    agin = dram.tile([4 * 128, BS], BF16, name="agin")
    agout = dram.tile([NCORES * 4 * 128, BS], BF16, name="agout",
                      addr_space="Shared")
    arin = dram.tile([1, 8], F32, name="arin")
    arout = dram.tile([1, 8], F32, name="arout", addr_space="Shared")

    # ---------- phase 1: normalize shards, local diag, transpose ----------
    with tc.tile_pool(name="nat", bufs=4) as natp, \
         tc.tile_pool(name="sq", bufs=3) as sqp, \
         tc.tile_pool(name="small", bufs=8) as smallp, \
         tc.tile_pool(name="nrm", bufs=4) as nrmp, \
         tc.tile_pool(name="tpsum", bufs=4, space="PSUM") as tpsum:
        for m in range(MT):
            r0 = m * 128
            nb = {}
            for src, key in ((t_in, "t"), (g_in, "g")):
                nat = natp.tile([128, D], F32, name="nat", tag="nat")
                nc.sync.dma_start(nat, src[r0:r0 + 128, :])
                sq = sqp.tile([128, D], F32, name="sq", tag="sq")
                ss = smallp.tile([128, 1], F32, name="ss", tag="ss")
                nc.scalar.activation(sq, nat, AF.Square, accum_out=ss)
                nrmv = smallp.tile([128, 1], F32, name="nrmv", tag="nrmv")
                nc.scalar.activation(nrmv, ss, AF.Sqrt)
                rn = smallp.tile([128, 1], F32, name="rn", tag="rn")
                nc.vector.reciprocal(rn, nrmv)
                nrm = nrmp.tile([128, D], BF16, name="nrm", tag="nrm")
                nc.vector.tensor_scalar_mul(nrm, nat, rn)
                nb[key] = nrm
            # local diagonal block: d_i = tn_i . gn_i (raw cosine, no temp)
            # (tensor_tensor_reduce crashes this runtime; use mult + reduce)
            dsc = sqp.tile([128, D], F32, name="dsc", tag="sq")
            nc.vector.tensor_tensor(dsc, nb["t"], nb["g"], OP.mult)
            nc.vector.reduce_sum(out=dbuf[:, m:m + 1], in_=dsc, axis=AX)
            for h in range(2):
                for nrm, dm in ((nb["t"], tdm), (nb["g"], gdm)):
                    pt = tpsum.tile([128, 128], BF16, name="pt", tag="pt")
                    nc.tensor.transpose(pt, nrm[:, h * 128:(h + 1) * 128],
                                        ident_bf)
                    nc.vector.tensor_copy(dm[h][:, r0:r0 + 128], pt)

    if mode == "prep1":
        with tc.tile_pool(name="ckp", bufs=1, space="PSUM") as ckp, \
             tc.tile_pool(name="cks", bufs=1) as cks:
            csum = cks.tile([128, 2], F32, name="csum")
            nc.vector.reduce_sum(out=csum[:, 0:1], in_=tdm[0], axis=AX)
            nc.vector.reduce_sum(out=csum[:, 1:2], in_=gdm[1], axis=AX)
            pchk = ckp.tile([1, 2], F32, name="pchk")
            nc.tensor.matmul(pchk, lhsT=ones_f32, rhs=csum, start=True,
                             stop=True)
            res = cks.tile([1, 5], F32, name="res")
            nc.vector.memset(res, 0.0)
            nc.vector.tensor_copy(res[0:1, 0:2], pchk[0:1, 0:2])
            nc.sync.dma_start(out5, res)
        return

    # ---------- phase 2: AllGather both modalities (d-major bf16) ----------
    if mode == "noag":
        for r in range(NCORES):
            for h in range(2):
                nc.sync.dma_start(tfull[h][:, r * BS:(r + 1) * BS], tdm[h])
                nc.sync.dma_start(gfull[h][:, r * BS:(r + 1) * BS], gdm[h])
    else:
        for h in range(2):
            nc.sync.dma_start(agin[h * 128:(h + 1) * 128, :], tdm[h])
            nc.sync.dma_start(agin[(2 + h) * 128:(3 + h) * 128, :], gdm[h])
        nc.gpsimd.collective_compute(
            "AllGather", OP.bypass, replica_groups=rg,
            ins=[agin.opt()], outs=[agout.opt()])
        for r in range(NCORES):
            base = r * 512
            for h in range(2):
                nc.sync.dma_start(tfull[h][:, r * BS:(r + 1) * BS],
                                  agout[base + h * 128:base + (h + 1) * 128, :])
                nc.sync.dma_start(gfull[h][:, r * BS:(r + 1) * BS],
                                  agout[base + 256 + h * 128:
                                        base + 256 + (h + 1) * 128, :])

    if mode == "prep":
        with tc.tile_pool(name="ckp", bufs=1, space="PSUM") as ckp, \
             tc.tile_pool(name="cks", bufs=1) as cks:
            csum = cks.tile([128, 2], F32, name="csum")
            nc.vector.reduce_sum(out=csum[:, 0:1], in_=tfull[0], axis=AX)
            nc.vector.reduce_sum(out=csum[:, 1:2], in_=gfull[1], axis=AX)
            pchk = ckp.tile([1, 2], F32, name="pchk")
            nc.tensor.matmul(pchk, lhsT=ones_f32, rhs=csum, start=True,
                             stop=True)
            res = cks.tile([1, 5], F32, name="res")
            nc.vector.memset(res, 0.0)
            nc.vector.tensor_copy(res[0:1, 0:2], pchk[0:1, 0:2])
            nc.sync.dma_start(out5, res)
        return

    # ---------- phase 3: two similarity slabs ----------
    with tc.tile_pool(name="spsum", bufs=8, space="PSUM") as spsum, \
         tc.tile_pool(name="escr", bufs=4) as escr, \
         tc.tile_pool(name="stats", bufs=4) as statp, \
         tc.tile_pool(name="slabacc", bufs=1) as slabp:
        for si, (lhs, rhs) in enumerate(((tdm, gfull), (gdm, tfull))):
            lseb = slabp.tile([128, MT], F32, name=f"lseb{si}")
            acb = slabp.tile([128, MT], F32, name=f"acb{si}")
            for m in range(MT):
                c0 = m * 128
                sums = statp.tile([128, NCH], F32, name="sums", tag="sums")
                maxb = statp.tile([128, NCH], F32, name="maxb", tag="maxb")
                for ci in range(NCH):
                    n0 = ci * NCHUNK
                    ps = spsum.tile([128, NCHUNK], F32, name="ps", tag="ps")
                    nc.tensor.matmul(ps, lhsT=lhs[0][:, c0:c0 + 128],
                                     rhs=rhs[0][:, n0:n0 + NCHUNK],
                                     start=True, stop=False)
                    nc.tensor.matmul(ps, lhsT=lhs[1][:, c0:c0 + 128],
                                     rhs=rhs[1][:, n0:n0 + NCHUNK],
                                     start=False, stop=True)
                    e = escr.tile([128, NCHUNK], F32, name="e", tag="e")
                    nc.scalar.activation(e, ps, AF.Exp, scale=INV_TEMP,
                                         accum_out=sums[:, ci:ci + 1])
                    nc.vector.reduce_max(out=maxb[:, ci:ci + 1], in_=ps,
                                         axis=AX)
                rs = statp.tile([128, 1], F32, name="rs", tag="rs")
                nc.vector.reduce_sum(out=rs, in_=sums, axis=AX)
                nc.scalar.activation(lseb[:, m:m + 1], rs, AF.Ln)
                rmx = statp.tile([128, 1], F32, name="rmx", tag="rmx")
                nc.vector.reduce_max(out=rmx, in_=maxb, axis=AX)
                nc.vector.tensor_tensor(acb[:, m:m + 1], dbuf[:, m:m + 1],
                                        rmx, OP.is_ge)
            nc.vector.reduce_sum(out=partials[:, si:si + 1], in_=lseb, axis=AX)
            nc.vector.reduce_sum(out=partials[:, 3 + si:4 + si], in_=acb,
                                 axis=AX)
        nc.vector.reduce_sum(out=partials[:, 2:3], in_=dbuf, axis=AX)

    # ---------- phase 4: cross-partition + cross-core reduction ----------
    with tc.tile_pool(name="rpsum", bufs=1, space="PSUM") as rpsum, \
         tc.tile_pool(name="fin", bufs=1) as finp:
        pps = rpsum.tile([1, 8], F32, name="pps")
        nc.tensor.matmul(pps[0:1, 0:5], lhsT=ones_f32, rhs=partials[:, 0:5],
                         start=True, stop=True)
        arin_s = finp.tile([1, 8], F32, name="arin_s")
        nc.vector.memset(arin_s, 0.0)
        nc.vector.tensor_copy(arin_s[0:1, 0:5], pps[0:1, 0:5])
        nc.sync.dma_start(arin, arin_s)
        nc.gpsimd.collective_compute(
            "AllReduce", OP.add, replica_groups=rg,
            ins=[arin.opt()], outs=[arout.opt()])
        tot = finp.tile([1, 8], F32, name="tot")
        nc.sync.dma_start(tot, arout)

        scaled = finp.tile([1, 8], F32, name="scaled")
        nc.vector.tensor_scalar_mul(scaled, tot, 1.0 / B)
        dterm = finp.tile([1, 1], F32, name="dterm")
        nc.vector.tensor_scalar_mul(dterm, scaled[0:1, 2:3], INV_TEMP)
        res = finp.tile([1, 5], F32, name="res")
        nc.vector.tensor_tensor(res[0:1, 1:2], scaled[0:1, 0:1], dterm,
                                OP.subtract)
        nc.vector.tensor_tensor(res[0:1, 2:3], scaled[0:1, 1:2], dterm,
                                OP.subtract)
        nc.vector.tensor_tensor(res[0:1, 0:1], res[0:1, 1:2], res[0:1, 2:3],
                                OP.add)
        nc.vector.tensor_copy(res[0:1, 3:5], scaled[0:1, 3:5])
        nc.sync.dma_start(out5, res)


_CACHE = {}


def build():
    if "nc" in _CACHE:
        return _CACHE["nc"]
    nc = bacc.Bacc("TRN2", target_bir_lowering=False, debug=False,
                   enable_asserts=False, num_devices=NCORES)
    t_in = nc.dram_tensor("t_shard", [BS, D], F32, kind="ExternalInput").ap()
    g_in = nc.dram_tensor("g_shard", [BS, D], F32, kind="ExternalInput").ap()
    out5 = nc.dram_tensor("out5", [1, 5], F32, kind="ExternalOutput").ap()
    import contextlib
    with tile.TileContext(nc) as tc:
        with contextlib.ExitStack() as ctx:
            emit(ctx, tc, t_in, g_in, out5)
    nc.compile()
    _CACHE["nc"] = nc
    return nc


def kernel(text_embeddings, graph_embeddings, **_):
    t = np.ascontiguousarray(np.asarray(text_embeddings, dtype=np.float32))
    g = np.ascontiguousarray(np.asarray(graph_embeddings, dtype=np.float32))
    assert t.shape == (B, D) and g.shape == (B, D)
    nc = build()
    in_maps = [
        {"t_shard": t[k * BS:(k + 1) * BS], "g_shard": g[k * BS:(k + 1) * BS]}
        for k in range(NCORES)
    ]
    res = run_bass_kernel_spmd(nc, in_maps, core_ids=list(range(NCORES)))
    return res.results[0]["out5"].reshape(5).astype(np.float32)


if __name__ == "__main__":
    rng = np.random.default_rng(0)
    t = rng.standard_normal((B, D), dtype=np.float32)
    g = rng.standard_normal((B, D), dtype=np.float32)
    print(kernel(text_embeddings=t, graph_embeddings=g))


# revision 10
# speedup vs baseline: 1.4267x; 1.4267x over previous
"""Distributed CLIP-style batch contrastive loss on 8 Trainium2 NeuronCores.

Math (B=8192, D=256, temperature=0.07):
    tn = l2norm(text), gn = l2norm(graph)
    sim = tn @ gn.T / temp                                  [B, B]
    t2g = mean(LSE_row(sim)) - mean(diag(sim))
    g2t = mean(LSE_col(sim)) - mean(diag(sim))
    acc = mean(diag is row/col argmax)
    out = [t2g+g2t, t2g, g2t, t2g_acc, g2t_acc]

Since |sim| <= 1/temp = 14.29, exp(sim) never overflows f32, so LSE is
computed without max subtraction:  LSE = log(sum(exp(x/temp))).

Sharding: each core owns a 1024-row shard of both modalities.  It
normalizes its shards (bf16, f32 accumulation), transposes them to
d-major via the PE, and AllGathers both modalities (g first so the
gather overlaps the t-side prep).  Each core then computes two
[1024, 8192] similarity slabs with bf16 matmuls (f32 PSUM):
  slab1 = tn_k @ gn_all.T   -> row stats    (t2g direction)
  slab2 = gn_k @ tn_all.T   -> column stats (g2t direction)
The ScalarE evaluates exp over 2048-wide PSUM spans with the activation
accumulator producing row partial sums; row maxes are taken in the exp
domain by VectorE tensor_scalar (max-accumulate) over the exp scratch in
SBUF (2x perf mode).  log() of the row sums is applied once at the end
(avoids Exp<->Ln activation-table swaps).  The diagonal block values are
computed locally from the natural-layout shards (t_i . g_i), which keeps
the whole program independent of the core id (pure SPMD).  Per-core
partial sums are AllReduced and every core assembles the final [5] vector.
"""

import numpy as np

import concourse.bacc as bacc
import concourse.bass as bass
import concourse.mybir as mybir
import concourse.tile as tile
from concourse import masks
from concourse.bass_utils import run_bass_kernel_spmd

B = 8192
D = 256
NCORES = 8
BS = B // NCORES          # 1024 rows per core
MT = BS // 128            # 8 m-tiles per shard
SPAN = 2048               # ACT exp span = 4 psum banks
NSP = B // SPAN           # 4 spans per slab row-tile
TEMP = 0.07
INV_TEMP = 1.0 / TEMP

F32 = mybir.dt.float32
BF16 = mybir.dt.bfloat16
AX = mybir.AxisListType.X
OP = mybir.AluOpType
AF = mybir.ActivationFunctionType


def emit(ctx, tc, t_in, g_in, out5, mode="full"):
    nc = tc.nc
    rg = [list(range(NCORES))]

    consts = ctx.enter_context(tc.tile_pool(name="consts", bufs=1))
    ident_bf = consts.tile([128, 128], BF16, name="ident_bf")
    masks.make_identity(nc, ident_bf)
    ones_f32 = consts.tile([128, 1], F32, name="ones_f32")
    nc.vector.memset(ones_f32, 1.0)

    # persistent SBUF tensors
    pers = ctx.enter_context(tc.tile_pool(name="pers", bufs=1))
    tdm = [pers.tile([128, BS], BF16, name=f"tdm{h}") for h in range(2)]
    gdm = [pers.tile([128, BS], BF16, name=f"gdm{h}") for h in range(2)]
    tfull = [pers.tile([128, B], BF16, name=f"tfull{h}") for h in range(2)]
    gfull = [pers.tile([128, B], BF16, name=f"gfull{h}") for h in range(2)]
    gball = pers.tile([128, MT * D], BF16, name="gball")  # normalized g rows
    dbuf = pers.tile([128, MT], F32, name="dbuf")         # raw diag dots
    edbuf = pers.tile([128, MT], F32, name="edbuf")       # exp(diag/temp)
    sums8 = [pers.tile([128, MT], F32, name=f"sums8_{s}") for s in range(2)]
    acb = [pers.tile([128, MT], F32, name=f"acb{s}") for s in range(2)]
    lseb = [pers.tile([128, MT], F32, name=f"lseb{s}") for s in range(2)]
    partials = pers.tile([128, 8], F32, name="partials")  # lse1 lse2 d a1 a2

    # DRAM bounce buffers for collectives
    dram = ctx.enter_context(tc.tile_pool(name="dram", bufs=1, space="DRAM"))
    agin_g = dram.tile([2 * 128, BS], BF16, name="agin_g")
    agout_g = dram.tile([NCORES * 2 * 128, BS], BF16, name="agout_g",
                        addr_space="Shared")
    agin_t = dram.tile([2 * 128, BS], BF16, name="agin_t")
    agout_t = dram.tile([NCORES * 2 * 128, BS], BF16, name="agout_t",
                        addr_space="Shared")
    arin = dram.tile([1, 8], F32, name="arin")
    arout = dram.tile([1, 8], F32, name="arout", addr_space="Shared")

    # ---------- phase 1: normalize shards, local diag, transpose ----------
    with tc.tile_pool(name="nat", bufs=4) as natp, \
         tc.tile_pool(name="sq", bufs=3) as sqp, \
         tc.tile_pool(name="small", bufs=8) as smallp, \
         tc.tile_pool(name="nrm", bufs=4) as nrmp, \
         tc.tile_pool(name="tpsum", bufs=4, space="PSUM") as tpsum:

        def normalize(src_slice, dst_bf):
            nat = natp.tile([128, D], F32, name="nat", tag="nat")
            nc.sync.dma_start(nat, src_slice)
            sq = sqp.tile([128, D], F32, name="sq", tag="sq")
            ss = smallp.tile([128, 1], F32, name="ss", tag="ss")
            nc.scalar.activation(sq, nat, AF.Square, accum_out=ss)
            nrmv = smallp.tile([128, 1], F32, name="nrmv", tag="nrmv")
            nc.scalar.activation(nrmv, ss, AF.Sqrt)
            rn = smallp.tile([128, 1], F32, name="rn", tag="rn")
            nc.vector.reciprocal(rn, nrmv)
            nc.vector.tensor_scalar_mul(dst_bf, nat, rn)

        def transpose_to(nrm_bf, dm, r0):
            for h in range(2):
                pt = tpsum.tile([128, 128], BF16, name="pt", tag="pt")
                nc.tensor.transpose(pt, nrm_bf[:, h * 128:(h + 1) * 128],
                                    ident_bf)
                nc.vector.tensor_copy(dm[h][:, r0:r0 + 128], pt)

        # g first so its AllGather flies while t is being prepped
        for m in range(MT):
            gb = gball[:, m * D:(m + 1) * D]
            normalize(g_in[m * 128:(m + 1) * 128, :], gb)
            transpose_to(gb, gdm, m * 128)

        for h in range(2):
            nc.sync.dma_start(agin_g[h * 128:(h + 1) * 128, :], gdm[h])
        nc.gpsimd.collective_compute(
            "AllGather", OP.bypass, replica_groups=rg,
            ins=[agin_g.opt()], outs=[agout_g.opt()])
        for r in range(NCORES):
            for h in range(2):
                nc.sync.dma_start(
                    gfull[h][:, r * BS:(r + 1) * BS],
                    agout_g[r * 256 + h * 128:r * 256 + (h + 1) * 128, :])

        for m in range(MT):
            tb = nrmp.tile([128, D], BF16, name="tb", tag="tb")
            normalize(t_in[m * 128:(m + 1) * 128, :], tb)
            # local diagonal block: d_i = tn_i . gn_i (raw cosine, no temp)
            dsc = sqp.tile([128, D], F32, name="dsc", tag="sq")
            nc.vector.tensor_tensor(dsc, tb, gball[:, m * D:(m + 1) * D],
                                    OP.mult)
            nc.vector.reduce_sum(out=dbuf[:, m:m + 1], in_=dsc, axis=AX)
            transpose_to(tb, tdm, m * 128)

        for h in range(2):
            nc.sync.dma_start(agin_t[h * 128:(h + 1) * 128, :], tdm[h])
        nc.gpsimd.collective_compute(
            "AllGather", OP.bypass, replica_groups=rg,
            ins=[agin_t.opt()], outs=[agout_t.opt()])
        for r in range(NCORES):
            for h in range(2):
                nc.sync.dma_start(
                    tfull[h][:, r * BS:(r + 1) * BS],
                    agout_t[r * 256 + h * 128:r * 256 + (h + 1) * 128, :])

    # exp(diag/temp) for the exp-domain argmax compare
    nc.scalar.activation(edbuf, dbuf, AF.Exp, scale=INV_TEMP)

    # ---------- phase 3: two similarity slabs ----------
    with tc.tile_pool(name="spsum", bufs=2, space="PSUM") as spsum, \
         tc.tile_pool(name="escr", bufs=3) as escr, \
         tc.tile_pool(name="mscr", bufs=2) as mscrp, \
         tc.tile_pool(name="stats", bufs=4) as statp:
        for si, (lhs, rhs) in enumerate(((tdm, gfull), (gdm, tfull))):
            for m in range(MT):
                c0 = m * 128
                sums4 = statp.tile([128, NSP], F32, name="sums4", tag="sums4")
                maxb4 = statp.tile([128, NSP], F32, name="maxb4", tag="maxb4")
                for sp in range(NSP):
                    ps = spsum.tile([128, SPAN], F32, name="ps", tag="ps")
                    for k in range(2):
                        for c4 in range(SPAN // 512):
                            n0 = sp * SPAN + c4 * 512
                            nc.tensor.matmul(
                                ps[:, c4 * 512:(c4 + 1) * 512],
                                lhsT=lhs[k][:, c0:c0 + 128],
                                rhs=rhs[k][:, n0:n0 + 512],
                                start=(k == 0), stop=(k == 1))
                    e = escr.tile([128, SPAN], F32, name="e", tag="e")
                    nc.scalar.activation(e, ps, AF.Exp, scale=INV_TEMP,
                                         accum_out=sums4[:, sp:sp + 1])
                    mscr = mscrp.tile([128, SPAN], BF16, name="mscr",
                                      tag="mscr")
                    nc.vector.tensor_scalar(mscr, e, 1.0, None, OP.mult,
                                            OP.max,
                                            accum_out=maxb4[:, sp:sp + 1])
                nc.vector.reduce_sum(out=sums8[si][:, m:m + 1], in_=sums4,
                                     axis=AX)
                rmx = statp.tile([128, 1], F32, name="rmx", tag="rmx")
                nc.vector.reduce_max(out=rmx, in_=maxb4, axis=AX)
                nc.vector.tensor_tensor(acb[si][:, m:m + 1],
                                        edbuf[:, m:m + 1], rmx, OP.is_ge)

        # batched log at the end (one Exp->Ln table swap total)
        for si in range(2):
            nc.scalar.activation(lseb[si], sums8[si], AF.Ln)
            nc.vector.reduce_sum(out=partials[:, si:si + 1], in_=lseb[si],
                                 axis=AX)
            nc.vector.reduce_sum(out=partials[:, 3 + si:4 + si], in_=acb[si],
                                 axis=AX)
        nc.vector.reduce_sum(out=partials[:, 2:3], in_=dbuf, axis=AX)

    # ---------- phase 4: cross-partition + cross-core reduction ----------
    with tc.tile_pool(name="rpsum", bufs=1, space="PSUM") as rpsum, \
         tc.tile_pool(name="fin", bufs=1) as finp:
        pps = rpsum.tile([1, 8], F32, name="pps")
        nc.tensor.matmul(pps[0:1, 0:5], lhsT=ones_f32, rhs=partials[:, 0:5],
                         start=True, stop=True)
        arin_s = finp.tile([1, 8], F32, name="arin_s")
        nc.vector.memset(arin_s, 0.0)
        nc.vector.tensor_copy(arin_s[0:1, 0:5], pps[0:1, 0:5])
        nc.sync.dma_start(arin, arin_s)
        nc.gpsimd.collective_compute(
            "AllReduce", OP.add, replica_groups=rg,
            ins=[arin.opt()], outs=[arout.opt()])
        tot = finp.tile([1, 8], F32, name="tot")
        nc.sync.dma_start(tot, arout)

        scaled = finp.tile([1, 8], F32, name="scaled")
        nc.vector.tensor_scalar_mul(scaled, tot, 1.0 / B)
        dterm = finp.tile([1, 1], F32, name="dterm")
        nc.vector.tensor_scalar_mul(dterm, scaled[0:1, 2:3], INV_TEMP)
        res = finp.tile([1, 5], F32, name="res")
        nc.vector.tensor_tensor(res[0:1, 1:2], scaled[0:1, 0:1], dterm,
                                OP.subtract)
        nc.vector.tensor_tensor(res[0:1, 2:3], scaled[0:1, 1:2], dterm,
                                OP.subtract)
        nc.vector.tensor_tensor(res[0:1, 0:1], res[0:1, 1:2], res[0:1, 2:3],
                                OP.add)
        nc.vector.tensor_copy(res[0:1, 3:5], scaled[0:1, 3:5])
        nc.sync.dma_start(out5, res)


_CACHE = {}


def build():
    if "nc" in _CACHE:
        return _CACHE["nc"]
    import contextlib
    nc = bacc.Bacc("TRN2", target_bir_lowering=False, debug=False,
                   enable_asserts=False, num_devices=NCORES)
    t_in = nc.dram_tensor("t_shard", [BS, D], F32, kind="ExternalInput").ap()
    g_in = nc.dram_tensor("g_shard", [BS, D], F32, kind="ExternalInput").ap()
    out5 = nc.dram_tensor("out5", [1, 5], F32, kind="ExternalOutput").ap()
    with tile.TileContext(nc) as tc:
        with contextlib.ExitStack() as ctx:
            emit(ctx, tc, t_in, g_in, out5)
    nc.compile()
    _CACHE["nc"] = nc
    return nc


def kernel(text_embeddings, graph_embeddings, **_):
    t = np.ascontiguousarray(np.asarray(text_embeddings, dtype=np.float32))
    g = np.ascontiguousarray(np.asarray(graph_embeddings, dtype=np.float32))
    assert t.shape == (B, D) and g.shape == (B, D)
    nc = build()
    in_maps = [
        {"t_shard": t[k * BS:(k + 1) * BS], "g_shard": g[k * BS:(k + 1) * BS]}
        for k in range(NCORES)
    ]
    res = run_bass_kernel_spmd(nc, in_maps, core_ids=list(range(NCORES)))
    return res.results[0]["out5"].reshape(5).astype(np.float32)


if __name__ == "__main__":
    rng = np.random.default_rng(0)
    t = rng.standard_normal((B, D), dtype=np.float32)
    g = rng.standard_normal((B, D), dtype=np.float32)
    print(kernel(text_embeddings=t, graph_embeddings=g))
